# revision 7
# baseline (speedup 1.0000x reference)
"""Trainium2 Bass kernel for one gated transformer block (B=2,T=2048,E=1024,H=16).

Sharding: core c = 4*b + q handles batch b, row-quarter q owning row-tiles
{q+4k : k=0..3} (128 rows each) with causal col extents Ek = 512*(k+1).
K/V compute is replicated across the 4 cores of a batch; all else is
token-parallel. Zero collectives; one SPMD NEFF. q-dependence is carried
entirely by per-core DATA (x_own gather + mask), instructions are uniform.
Activations live transposed [feature, token]; scores are built natural [t,s]
for softmax + HBM write and PE-transposed for the AV matmul.
"""
import sys
sys.path.insert(0, '/opt/trn_rl_repo')
import contextlib
import numpy as np
import ml_dtypes

import concourse.bass as bass
from concourse import bacc
import concourse.tile as tile
from concourse import mybir
from concourse.bass_utils import run_bass_kernel_spmd
from concourse.masks import make_identity

F32 = mybir.dt.float32
F32R = mybir.dt.float32r
BF16 = mybir.dt.bfloat16
AF = mybir.ActivationFunctionType
ALU = mybir.AluOpType
AX = mybir.AxisListType

B, T, E, H, HS = 2, 2048, 1024, 16, 64
EPS = 1e-5
NEG = float(np.finfo(np.float32).min)

_CACHE = {}


def _ln_stats(nc, pools, xn):
    """Natural-layout LN stats for [128, E] tile -> (mu, rstd) [128,1] f32."""
    stp = pools["stp"]
    stats = stp.tile([128, 2, nc.vector.BN_STATS_DIM], F32, tag="bnst")
    for sg in range(2):
        nc.vector.bn_stats(out=stats[:, sg, :], in_=xn[:, sg * 512:(sg + 1) * 512])
    mv = stp.tile([128, nc.vector.BN_AGGR_DIM], F32, tag="bnmv")
    nc.vector.bn_aggr(out=mv[:, :], in_=stats[:, :, :])
    rstd = stp.tile([128, 1], F32, tag="rstd")
    nc.scalar.activation(out=rstd[:, :], in_=mv[:, 1:2], func=AF.Sqrt,
                         bias=pools["eps_col"][:, :], scale=1.0)
    nc.vector.reciprocal(out=rstd[:, :], in_=rstd[:, :])
    return mv[:, 0:1], rstd


def _build():
    nc = bacc.Bacc("TRN2", target_bir_lowering=False, debug=False, num_devices=8)
    dp = nc.declare_dram_parameter
    xb = dp("xb", [T, E], F32, isOutput=False)
    x_own = dp("x_own", [512, E], F32, isOutput=False)
    mask = dp("mask", [128, 512], F32, isOutput=False)
    wq_r = dp("wq_r", [E, E], BF16, isOutput=False)
    wk_r = dp("wk_r", [E, E], BF16, isOutput=False)
    wv_r = dp("wv_r", [E, E], BF16, isOutput=False)
    wp_d = dp("wp", [E, E], F32, isOutput=False)
    wg_att_d = dp("wg_att", [E, E], F32, isOutput=False)
    w1_d = dp("w1", [E, 4 * E], F32, isOutput=False)
    w2_d = dp("w2", [4 * E, 2 * E], BF16, isOutput=False)
    w3_d = dp("w3", [2 * E, E], F32, isOutput=False)
    wg_ff_d = dp("wg_ff", [E, E], F32, isOutput=False)
    vb = {}
    for nm, n in [("bp", E), ("bg_att", E), ("b1", 4 * E), ("b2", 2 * E),
                  ("b3", E), ("bg_ff", E), ("ln1_g", E), ("ln1_b", E),
                  ("ln2_g", E), ("ln2_b", E)]:
        vb[nm] = dp(nm, [n], F32, isOutput=False)
    wei_o = dp("wei", [H, 4, 128, T], F32, isOutput=True)
    out_o = dp("out", [4, 128, E], F32, isOutput=True)

    with tile.TileContext(nc) as tc, contextlib.ExitStack() as ctx:
        P = lambda name, bufs: ctx.enter_context(tc.tile_pool(name=name, bufs=bufs))
        consts = P("consts", 1)
        big = P("big", 8)        # hT bf16 [128,2048] -> f1T bf16 [128,4,512]
        hop = P("hop", 8)        # hT_own bf16 [128,512]
        vpool = P("vpool", 16)   # V pair-cols f32r [128,128]
        f2p = P("f2p", 16)       # f2T f32r [128,512]
        ktp = P("ktp", 2)        # kt f32r [128,2048] -> x1T f32r [128,4,512]
        qtp = P("qtp", 2)        # qt f32r [128,512]
        x2p = P("x2p", 8)        # x2T f32r [128,512]
        attnp = P("attnp", 8)    # attnT f32r [128,512] -> f3T f32r
        sap = P("sap", 8)        # saT f32r [128,512] -> g2T f32
        gp = P("gp", 1)          # gate tmp f32 [128,512]
        repp = P("repp", 2)      # mu_rep / rstd_rep f32 [128,512]
        sqp = P("sqp", 2)        # sq / tmp f32r [128,512]
        weip = [P(f"wei{k}", 1) for k in range(4)]
        wtp = P("wtp", 1)        # weiT f32r [128,512]
        wts = P("wts", 2)        # weight stream [128,<=2048]
        xnp = P("xnp", 1)        # x natural stream f32 [128,1024]
        stp = P("stp", 4)
        ps = ctx.enter_context(tc.tile_pool(name="ps", bufs=8, space="PSUM"))

        # ---- constants ----
        ident = consts.tile([128, 128], F32)
        make_identity(nc, ident[:, :])
        mask_sb = consts.tile([128, 512], F32)
        nc.sync.dma_start(out=mask_sb[:, :], in_=mask[:, :])
        cols = {}
        for nm in vb:
            n = vb[nm].shape[0]
            t_ = consts.tile([128, n // 128], F32, name=f"col_{nm}")
            nc.sync.dma_start(out=t_[:, :], in_=vb[nm].ap().rearrange("(c p) -> p c", p=128))
            cols[nm] = t_
        onesf = consts.tile([128, 1], F32)
        nc.vector.memset(onesf[:, :], 1.0)
        ones_col = consts.tile([128, 1], F32R)
        nc.vector.tensor_copy(out=ones_col[:, :], in_=onesf[:, :])
        ones_rowf = consts.tile([1, 128], F32)
        nc.vector.memset(ones_rowf[:, :], 1.0)
        ones_row = consts.tile([1, 128], F32R)
        nc.vector.tensor_copy(out=ones_row[:, :], in_=ones_rowf[:, :])
        eps_col = consts.tile([128, 1], F32)
        nc.vector.memset(eps_col[:, :], EPS)
        shiftf = consts.tile([64, 128], F32)
        nc.vector.memset(shiftf[:, :], 0.0)
        nc.vector.tensor_copy(out=shiftf[:, 64:128], in_=ident[0:64, 0:64])
        shift_r = consts.tile([64, 128], F32R)
        nc.vector.tensor_copy(out=shift_r[:, :], in_=shiftf[:, :])
        pools = {"stp": stp, "eps_col": eps_col}

        # ---- phase 0: hT (bf16, LN1'd, transposed) from xb; hT_own from x_own ----
        hT = [big.tile([128, T], BF16, tag="big", name=f"hT{i}") for i in range(8)]
        hT_own = [hop.tile([128, 512], BF16, tag="hop", name=f"hTo{i}") for i in range(8)]

        def ln1_transpose(src_ap, ntiles, dst, dstoff):
            for i in range(ntiles):
                xn = xnp.tile([128, E], F32, tag="xn")
                nc.sync.dma_start(out=xn[:, :], in_=src_ap[i * 128:(i + 1) * 128, :])
                mu, rstd = _ln_stats(nc, pools, xn)
                xc = xnp.tile([128, E], F32, tag="xc")
                nc.vector.tensor_scalar(out=xc[:, :], in0=xn[:, :], scalar1=mu,
                                        scalar2=rstd, op0=ALU.subtract, op1=ALU.mult)
                for eh in range(2):  # transpose 8 e-chunks via 2 psum banks
                    pt = ps.tile([128, 512], F32, tag="ps")
                    for e4 in range(4):
                        e = eh * 4 + e4
                        nc.tensor.transpose(pt[:, e4 * 128:(e4 + 1) * 128],
                                            xc[:, e * 128:(e + 1) * 128], ident[:, :])
                    for e4 in range(4):
                        e = eh * 4 + e4
                        nc.vector.tensor_scalar(
                            out=dst[e][:, dstoff + i * 128: dstoff + (i + 1) * 128],
                            in0=pt[:, e4 * 128:(e4 + 1) * 128],
                            scalar1=cols["ln1_g"][:, e:e + 1],
                            scalar2=cols["ln1_b"][:, e:e + 1],
                            op0=ALU.mult, op1=ALU.add)

        ln1_transpose(xb.ap(), 16, hT, 0)
        ln1_transpose(x_own.ap(), 4, hT_own, 0)

        # ---- attention ----
        attnT = [attnp.tile([128, 512], F32R, tag="attn", name=f"attnT{i}") for i in range(8)]
        sc2 = float(HS) ** -0.5

        for p in range(8):  # head pairs
            # V pair-cols: V_p[j] [128(s),128(2 heads)] f32r
            Vp = []
            wvt = wts.tile([128, 8, 128], BF16, tag="wslice")
            nc.sync.dma_start(out=wvt[:, :, :],
                              in_=wv_r.ap().rearrange("(c p) n -> p c n", p=128)[:, :, p * 128:(p + 1) * 128])
            for j in range(16):
                pv = ps.tile([128, 512], F32, tag="ps")
                for e in range(8):
                    nc.tensor.matmul(pv[:, 0:128], hT[e][:, j * 128:(j + 1) * 128],
                                     wvt[:, e, :], start=(e == 0), stop=(e == 7))
                vt = vpool.tile([128, 128], F32R, tag="v")
                nc.vector.tensor_copy(out=vt[:, :], in_=pv[:, 0:128])
                Vp.append(vt)
            # kt[p]: [128(2h dims), T] f32r
            kt = ktp.tile([128, T], F32R, tag="kt")
            wkt = wts.tile([128, 8, 128], BF16, tag="wslice")
            nc.sync.dma_start(out=wkt[:, :, :],
                              in_=wk_r.ap().rearrange("(c p) n -> p c n", p=128)[:, :, p * 128:(p + 1) * 128])
            for j in range(4):
                pk = ps.tile([128, 512], F32, tag="ps")
                for e in range(8):
                    nc.tensor.matmul(pk[:, :], wkt[:, e, :],
                                     hT[e][:, j * 512:(j + 1) * 512],
                                     start=(e == 0), stop=(e == 7))
                nc.vector.tensor_copy(out=kt[:, j * 512:(j + 1) * 512], in_=pk[:, :])
            # qt[p]: [128, 512(own)] f32r, scaled HS^-0.5
            qt = qtp.tile([128, 512], F32R, tag="qt")
            wqt = wts.tile([128, 8, 128], BF16, tag="wslice")
            nc.sync.dma_start(out=wqt[:, :, :],
                              in_=wq_r.ap().rearrange("(c p) n -> p c n", p=128)[:, :, p * 128:(p + 1) * 128])
            pq = ps.tile([128, 512], F32, tag="ps")
            for e in range(8):
                nc.tensor.matmul(pq[:, :], wqt[:, e, :],
                                 hT_own[e][:, :], start=(e == 0), stop=(e == 7))
            nc.vector.tensor_scalar_mul(qt[:, :], pq[:, :], sc2)

            for h2 in range(2):
                h = 2 * p + h2
                sl = slice(64 * h2, 64 * h2 + 64)
                wei_sb = []
                for k in range(4):
                    Ek = 512 * (k + 1)
                    wsb = weip[k].tile([128, Ek], F32, tag=f"w{k}", name=f"wsb{k}")
                    for j in range(k + 1):
                        psc = ps.tile([128, 512], F32, tag="ps")
                        nc.tensor.matmul(psc[:, :], qt[sl, k * 128:(k + 1) * 128],
                                         kt[sl, j * 512:(j + 1) * 512],
                                         start=True, stop=True)
                        if j == k:
                            nc.vector.tensor_tensor(wsb[:, j * 512:(j + 1) * 512],
                                                    mask_sb[:, :], psc[:, :], ALU.add)
                        else:
                            nc.vector.tensor_copy(out=wsb[:, j * 512:(j + 1) * 512],
                                                  in_=psc[:, :])
                    mneg = stp.tile([128, 1], F32, tag="mneg")
                    nc.vector.tensor_reduce(out=mneg[:, :], in_=wsb[:, :], axis=AX.X,
                                            op=ALU.max, negate=True)
                    ssum = stp.tile([128, 1], F32, tag="ssum")
                    nc.scalar.activation(out=wsb[:, :], in_=wsb[:, :], func=AF.Exp,
                                         bias=mneg[:, :], accum_out=ssum[:, :])
                    rinv = stp.tile([128, 1], F32, tag="rinv")
                    nc.vector.reciprocal(out=rinv[:, :], in_=ssum[:, :])
                    nc.vector.tensor_scalar_mul(wsb[:, :], wsb[:, :], rinv)
                    nc.sync.dma_start(out=wei_o.ap()[h, k, :, 0:Ek], in_=wsb[:, :])
                    wei_sb.append(wsb)
                # AV: accumulate attnT_h [64, 512] over 16 s-chunks
                pa = ps.tile([128, 512], F32, tag="ps")
                for j in range(16):
                    k0 = j // 4
                    wT = wtp.tile([128, 512], F32R, tag="wT")
                    ptr = ps.tile([128, 512], F32, tag="ps")
                    for k in range(k0, 4):
                        nc.tensor.transpose(ptr[:, (k - k0) * 128:(k - k0 + 1) * 128],
                                            wei_sb[k][:, j * 128:(j + 1) * 128],
                                            ident[:, :])
                    ncols = (4 - k0) * 128
                    nc.vector.tensor_copy(out=wT[:, 0:ncols], in_=ptr[:, 0:ncols])
                    nc.tensor.matmul(pa[0:64, k0 * 128:512], Vp[j][:, sl],
                                     wT[:, 0:ncols], start=(j == 0), stop=(j == 15))
                if h2 == 0:
                    nc.vector.tensor_copy(out=attnT[p][0:64, :], in_=pa[0:64, :])
                else:
                    tmp = wtp.tile([64, 512], F32R, tag="wT", name="avtmp")
                    nc.vector.tensor_copy(out=tmp[:, :], in_=pa[0:64, :])
                    pshift = ps.tile([128, 512], F32, tag="ps")
                    nc.tensor.matmul(pshift[:, :], shift_r[:, :], tmp[:, :],
                                     start=True, stop=True)
                    nc.vector.tensor_copy(out=attnT[p][64:128, :], in_=pshift[64:128, :])

        # ---- phase 3: saT = wp.T @ attnT + bp ; gate ; x1T ----
        saT = [sap.tile([128, 512], F32R, tag="sa", name=f"saT{i}") for i in range(8)]
        psa = [ps.tile([128, 512], F32, tag="ps", name=f"psa{i}") for i in range(8)]
        for hk in range(8):
            wpt = wts.tile([128, E], F32R, tag="wbig")
            nc.sync.dma_start(out=wpt[:, :],
                              in_=wp_d.ap().bitcast(F32R).rearrange("(c p) n -> p c n", p=128)[:, hk, :])
            for eo in range(8):
                nc.tensor.matmul(psa[eo][:, :], wpt[:, eo * 128:(eo + 1) * 128],
                                 attnT[hk][:, :], start=(hk == 0), stop=(hk == 7))
        for eo in range(8):
            nc.vector.tensor_scalar(out=saT[eo][:, :], in0=psa[eo][:, :],
                                    scalar1=cols["bp"][:, eo:eo + 1], scalar2=None,
                                    op0=ALU.add)
        # x1T pre-load with xT (raw x_own transposed)
        x1T = [ktp.tile([128, 4, 512], F32R, tag="kt", name=f"x1T{i}") for i in range(2)]

        def x1t_ap(e):
            return x1T[e // 4][:, e % 4, :]

        for k in range(4):
            xn = xnp.tile([128, E], F32, tag="xn")
            nc.sync.dma_start(out=xn[:, :], in_=x_own.ap()[k * 128:(k + 1) * 128, :])
            for eh in range(2):
                pt = ps.tile([128, 512], F32, tag="ps")
                for e4 in range(4):
                    e = eh * 4 + e4
                    nc.tensor.transpose(pt[:, e4 * 128:(e4 + 1) * 128],
                                        xn[:, e * 128:(e + 1) * 128], ident[:, :])
                for e4 in range(4):
                    e = eh * 4 + e4
                    nc.vector.tensor_copy(out=x1t_ap(e)[:, k * 128:(k + 1) * 128],
                                          in_=pt[:, e4 * 128:(e4 + 1) * 128])
        # gate matmuls
        pga = [ps.tile([128, 512], F32, tag="ps", name=f"pga{i}") for i in range(8)]
        for ei in range(8):
            wgt = wts.tile([128, E], F32R, tag="wbig")
            nc.sync.dma_start(out=wgt[:, :],
                              in_=wg_att_d.ap().bitcast(F32R).rearrange("(c p) n -> p c n", p=128)[:, ei, :])
            for eo in range(8):
                nc.tensor.matmul(pga[eo][:, :], wgt[:, eo * 128:(eo + 1) * 128],
                                 saT[ei][:, :], start=(ei == 0), stop=(ei == 7))
        for eo in range(8):
            gt = gp.tile([128, 512], F32, tag="g")
            nc.scalar.activation(out=gt[:, :], in_=pga[eo][:, :], func=AF.Sigmoid,
                                 bias=cols["bg_att"][:, eo:eo + 1])
            nc.vector.tensor_tensor(x1t_ap(eo), gt[:, :], x1t_ap(eo), ALU.mult)
            nc.vector.tensor_tensor(x1t_ap(eo), saT[eo][:, :], x1t_ap(eo), ALU.add)

        # ---- phase 4: LN2 via matmul stats ----
        ps1 = ps.tile([128, 512], F32, tag="ps")
        ps2 = ps.tile([128, 512], F32, tag="ps")
        for e in range(8):
            sq = sqp.tile([128, 512], F32R, tag="sq", name=f"sq{e}")
            nc.vector.tensor_tensor(sq[:, :], x1t_ap(e), x1t_ap(e), ALU.mult)
            nc.tensor.matmul(ps1[0:1, :], ones_col[:, :], x1t_ap(e),
                             start=(e == 0), stop=(e == 7))
            nc.tensor.matmul(ps2[0:1, :], ones_col[:, :], sq[:, :],
                             start=(e == 0), stop=(e == 7))
        s1 = sqp.tile([1, 512], F32R, tag="sq", name="s1")
        s2 = sqp.tile([1, 512], F32R, tag="sq", name="s2")
        nc.vector.tensor_copy(out=s1[:, :], in_=ps1[0:1, :])
        nc.vector.tensor_copy(out=s2[:, :], in_=ps2[0:1, :])
        pm1 = ps.tile([128, 512], F32, tag="ps")
        pm2 = ps.tile([128, 512], F32, tag="ps")
        nc.tensor.matmul(pm1[:, :], ones_row[:, :], s1[:, :], start=True, stop=True)
        nc.tensor.matmul(pm2[:, :], ones_row[:, :], s2[:, :], start=True, stop=True)
        mu_rep = repp.tile([128, 512], F32, tag="rep")
        rstd_rep = repp.tile([128, 512], F32, tag="rep")
        nc.vector.tensor_scalar_mul(mu_rep[:, :], pm1[:, :], 1.0 / E)
        mu2 = sqp.tile([128, 512], F32, tag="sq", name="mu2")
        nc.vector.tensor_tensor(mu2[:, :], mu_rep[:, :], mu_rep[:, :], ALU.mult)
        var = sqp.tile([128, 512], F32, tag="sq", name="var")
        nc.vector.tensor_scalar_mul(var[:, :], pm2[:, :], 1.0 / E)
        nc.vector.tensor_tensor(var[:, :], var[:, :], mu2[:, :], ALU.subtract)
        nc.scalar.activation(out=rstd_rep[:, :], in_=var[:, :], func=AF.Sqrt,
                             bias=eps_col[:, :], scale=1.0)
        nc.vector.reciprocal(out=rstd_rep[:, :], in_=rstd_rep[:, :])
        x2T = [x2p.tile([128, 512], F32R, tag="x2", name=f"x2T{i}") for i in range(8)]
        for e in range(8):
            nc.vector.tensor_tensor(x2T[e][:, :], x1t_ap(e), mu_rep[:, :], ALU.subtract)
            nc.vector.tensor_tensor(x2T[e][:, :], x2T[e][:, :], rstd_rep[:, :], ALU.mult)
            nc.vector.tensor_scalar(out=x2T[e][:, :], in0=x2T[e][:, :],
                                    scalar1=cols["ln2_g"][:, e:e + 1],
                                    scalar2=cols["ln2_b"][:, e:e + 1],
                                    op0=ALU.mult, op1=ALU.add)

        # ---- phase 5: FFN ----
        f1T = [big.tile([128, 4, 512], BF16, tag="big", name=f"f1T{i}") for i in range(8)]

        def f1_ap(o):
            return f1T[o // 4][:, o % 4, :]

        for og in range(4):  # 8 out-chunks per group
            pf = [ps.tile([128, 512], F32, tag="ps", name=f"pf{i}") for i in range(8)]
            for ei in range(8):
                w1t = wts.tile([128, 1024], F32R, tag="wbig")
                nc.sync.dma_start(out=w1t[:, :],
                                  in_=w1_d.ap().bitcast(F32R).rearrange("(c p) n -> p c n", p=128)[:, ei, og * 1024:(og + 1) * 1024])
                for o8 in range(8):
                    nc.tensor.matmul(pf[o8][:, :], w1t[:, o8 * 128:(o8 + 1) * 128],
                                     x2T[ei][:, :], start=(ei == 0), stop=(ei == 7))
            for o8 in range(8):
                o = og * 8 + o8
                nc.scalar.activation(out=f1_ap(o), in_=pf[o8][:, :], func=AF.Gelu,
                                     bias=cols["b1"][:, o:o + 1])
        f2T = [f2p.tile([128, 512], F32R, tag="f2", name=f"f2T{i}") for i in range(16)]
        for og in range(2):
            pf = [ps.tile([128, 512], F32, tag="ps", name=f"pf{i}") for i in range(8)]
            for ei in range(32):
                w2t = wts.tile([128, 1024], BF16, tag="wbig")
                nc.sync.dma_start(out=w2t[:, :],
                                  in_=w2_d.ap().rearrange("(c p) n -> p c n", p=128)[:, ei, og * 1024:(og + 1) * 1024])
                for o8 in range(8):
                    nc.tensor.matmul(pf[o8][:, :], w2t[:, o8 * 128:(o8 + 1) * 128],
                                     f1_ap(ei), start=(ei == 0), stop=(ei == 31))
            for o8 in range(8):
                o = og * 8 + o8
                nc.scalar.activation(out=f2T[o][:, :], in_=pf[o8][:, :], func=AF.Gelu,
                                     bias=cols["b2"][:, o:o + 1])
        f3T = [attnp.tile([128, 512], F32R, tag="attn", name=f"f3T{i}") for i in range(8)]
        pf = [ps.tile([128, 512], F32, tag="ps", name=f"pfb{i}") for i in range(8)]
        for ei in range(16):
            w3t = wts.tile([128, 1024], F32R, tag="wbig")
            nc.sync.dma_start(out=w3t[:, :],
                              in_=w3_d.ap().bitcast(F32R).rearrange("(c p) n -> p c n", p=128)[:, ei, :])
            for o8 in range(8):
                nc.tensor.matmul(pf[o8][:, :], w3t[:, o8 * 128:(o8 + 1) * 128],
                                 f2T[ei][:, :], start=(ei == 0), stop=(ei == 15))
        for o8 in range(8):
            nc.vector.tensor_scalar(out=f3T[o8][:, :], in0=pf[o8][:, :],
                                    scalar1=cols["b3"][:, o8:o8 + 1], scalar2=None,
                                    op0=ALU.add)
        pf = [ps.tile([128, 512], F32, tag="ps", name=f"pfb{i}") for i in range(8)]
        for ei in range(8):
            wgt = wts.tile([128, E], F32R, tag="wbig")
            nc.sync.dma_start(out=wgt[:, :],
                              in_=wg_ff_d.ap().bitcast(F32R).rearrange("(c p) n -> p c n", p=128)[:, ei, :])
            for o8 in range(8):
                nc.tensor.matmul(pf[o8][:, :], wgt[:, o8 * 128:(o8 + 1) * 128],
                                 f3T[ei][:, :], start=(ei == 0), stop=(ei == 7))
        outT = [sap.tile([128, 512], F32, tag="sa", name=f"outT{i}") for i in range(8)]
        for o8 in range(8):
            g2 = gp.tile([128, 512], F32, tag="g")
            nc.scalar.activation(out=g2[:, :], in_=pf[o8][:, :], func=AF.Sigmoid,
                                 bias=cols["bg_ff"][:, o8:o8 + 1])
            nc.vector.tensor_tensor(outT[o8][:, :], g2[:, :], x1t_ap(o8), ALU.mult)
            nc.vector.tensor_tensor(outT[o8][:, :], outT[o8][:, :], f3T[o8][:, :], ALU.add)

        # ---- phase 6: transpose back + write ----
        for k in range(4):
            on = xnp.tile([128, E], F32, tag="xn")
            for eh in range(2):
                pt = ps.tile([128, 512], F32, tag="ps")
                for e4 in range(4):
                    e = eh * 4 + e4
                    nc.tensor.transpose(pt[:, e4 * 128:(e4 + 1) * 128],
                                        outT[e][:, k * 128:(k + 1) * 128], ident[:, :])
                nc.vector.tensor_copy(out=on[:, eh * 512:(eh + 1) * 512], in_=pt[:, :])
            nc.sync.dma_start(out=out_o.ap()[k, :, :], in_=on[:, :])

    nc.compile()
    return nc


def _host_prep(inputs):
    i = {k: np.ascontiguousarray(np.asarray(v, dtype=np.float32)) for k, v in inputs.items()}
    wq_r = np.ascontiguousarray(i['wq'].transpose(1, 0, 2).reshape(E, E)).astype(ml_dtypes.bfloat16)
    wk_r = np.ascontiguousarray(i['wk'].transpose(1, 0, 2).reshape(E, E)).astype(ml_dtypes.bfloat16)
    wv_r = np.ascontiguousarray(i['wv'].transpose(1, 0, 2).reshape(E, E)).astype(ml_dtypes.bfloat16)
    w2b = i['w2'].astype(ml_dtypes.bfloat16)
    masks = []
    r = np.arange(128)[:, None]
    c = np.arange(512)[None, :]
    for q in range(4):
        masks.append(np.where(c <= 128 * q + r, np.float32(0), np.float32(NEG)).astype(np.float32))
    shared = dict(wq_r=wq_r, wk_r=wk_r, wv_r=wv_r, w2=w2b,
                  wp=i['wp'], wg_att=i['wg_att'], w1=i['w1'], w3=i['w3'],
                  wg_ff=i['wg_ff'], bp=i['bp'], bg_att=i['bg_att'], b1=i['b1'],
                  b2=i['b2'], b3=i['b3'], bg_ff=i['bg_ff'], ln1_g=i['ln1_g'],
                  ln1_b=i['ln1_b'], ln2_g=i['ln2_g'], ln2_b=i['ln2_b'])
    in_maps = []
    for cid in range(8):
        b, q = cid // 4, cid % 4
        ownrows = np.concatenate([i['x'][b][(q + 4 * k) * 128:(q + 4 * k + 1) * 128]
                                  for k in range(4)], axis=0)
        m = dict(shared)
        m['xb'] = i['x'][b]
        m['x_own'] = np.ascontiguousarray(ownrows)
        m['mask'] = masks[q]
        in_maps.append(m)
    return in_maps


def kernel(**inputs):
    if 'nc' not in _CACHE:
        _CACHE['nc'] = _build()
    nc = _CACHE['nc']
    in_maps = _host_prep(inputs)
    res = run_bass_kernel_spmd(nc, in_maps, list(range(8)))
    wei_full = np.zeros((B, H, T, T), np.float32)
    out_full = np.zeros((B, T, E), np.float32)
    for cid in range(8):
        b, q = cid // 4, cid % 4
        wei_p = res.results[cid]["wei"]
        out_p = res.results[cid]["out"]
        for k in range(4):
            t0 = (q + 4 * k) * 128
            wei_full[b, :, t0:t0 + 128, :] = wei_p[:, k]
            out_full[b, t0:t0 + 128, :] = out_p[k]
    return out_full, wei_full
